# revision 18
# baseline (speedup 1.0000x reference)
"""Multi-head attention + output projection on 8 Trainium2 NeuronCores.

Problem (hardcoded): x [3, 2, 4096, 512] fp32 (q/k/v stacked), proj_w [512, 512],
proj_b [512].  reference = softmax(q k^T / sqrt(64)) v, heads=8, then
out @ proj_w.T + proj_b.

Sharding: B*H = 16 (batch, head) pairs over 8 cores -> each core gets one
batch and one adjacent head PAIR (2 heads = 128 feature dims).  The output
projection is tensor-parallel along the contraction dim: each core computes
its partial y = attn_out_pair @ W[:, pair_dims].T; the host sums the 4
partials per batch and adds the bias.

Device kernel (per core).  All matmul operands are bf16, PSUM accumulation
fp32:
  scores^T[nk, nq] = kT_chunk.T @ qT      (two heads row-tiled in the PE
                                           array: K=64 each at rows 0-63 /
                                           64-127, concurrent)
  P^T: split across two engines per chunk —
    ScalarE chunks:  pt = exp(0.125 * scores^T) -> bf16   (native ACT exp)
    VectorE chunks:  u  = int16(round(A*scores^T + B)); the int16 bit
      pattern IS the bf16 encoding of exp(0.125*s) (Schraudolph trick,
      ~±3% sawtooth, zero-mean calibrated); PV reads u.bitcast(bf16).
  acc[d, nq]  += [V | 1].T @ P^T          (K=128; row 64 = softmax denom)
  y_h[nq, o]   = st_h.T @ W_h  as a row-tiled concurrent pair (h0 rows
                 0-63, h1 rows 64-127), shipped to the host UN-normalized
                 together with the denominator rows; the host applies
                 y = y_h0/den_h0 + y_h1/den_h1 (fp32).  This deletes the
                 den-transpose DMA round trip and the ScalarE/VectorE
                 normalize chain from the device critical path.
"""

import numpy as np

C, B, N, D, H = 3, 2, 4096, 512, 8
HD = 64          # head dim
NCORES = 8
NQB = 512        # nq block width (PSUM bank)
NBLK = N // NQB  # 8 nq blocks
NCHUNK = N // 128  # 32 nk chunks of 128

# Schraudolph constants: bf16 bits of exp(0.125*s) ~ round(A*s + B).
SCH_A = float(16.0 / np.log(2.0))
SCH_B = float(128.0 * 127.0 - 7.36)
# chunks whose exp runs on the DVE (rest on ScalarE); spread evenly.
DVE_CKS = frozenset({2, 4, 6, 9, 11, 14, 16, 19, 21, 24, 26, 28, 30})

_compiled = None


def _build_nc():
    import concourse.bacc as bacc
    import concourse.tile as tile
    from concourse import mybir

    f32 = mybir.dt.float32
    bf16 = mybir.dt.bfloat16
    i16 = mybir.dt.int16
    Exp = mybir.ActivationFunctionType.Exp
    Copy = mybir.ActivationFunctionType.Copy
    mult = mybir.AluOpType.mult
    add = mybir.AluOpType.add

    nc = bacc.Bacc("TRN2", target_bir_lowering=False, debug=False, num_devices=1)

    qT = nc.dram_tensor("qT", [128, N], bf16, kind="ExternalInput").ap()
    kT = nc.dram_tensor("kT", [128, N], bf16, kind="ExternalInput").ap()
    vI = nc.dram_tensor("vI", [128, NCHUNK, 2, HD + 1], bf16, kind="ExternalInput").ap()
    stout = nc.dram_tensor("stout", [NBLK, 128, NQB], bf16,
                           kind="ExternalOutput").ap()
    dno = nc.dram_tensor("dno", [NBLK, 2 * NQB], f32, kind="ExternalOutput").ap()

    with tile.TileContext(nc) as tc:
        with (
            tc.tile_pool(name="const", bufs=1) as const_pool,
            tc.tile_pool(name="pt", bufs=5) as pt_pool,
            tc.tile_pool(name="ep", bufs=3) as ep_pool,
            tc.tile_pool(name="ps_s", bufs=3, space="PSUM") as ps_s,
            tc.tile_pool(name="ps_a", bufs=1, space="PSUM") as ps_a,
        ):
            # resident inputs
            qT_sb = const_pool.tile([128, N], bf16)
            kT_sb = const_pool.tile([128, N], bf16)
            vI_sb = const_pool.tile([128, NCHUNK, 2, HD + 1], bf16)
            # zero weights for the HAM-warming filler matmuls first so the
            # PE can start warming at t~0, then trigger the exp table load
            # while the input DMAs stream in
            zeros_sb = const_pool.tile([128, NQB], mybir.dt.bfloat16)
            nc.vector.memset(zeros_sb[:], 0.0)
            warm = ep_pool.tile([128, 2], f32, tag="warm")
            nc.vector.memset(warm[:], 0.0)
            nc.scalar.activation(warm[:, 1:2], warm[:, 0:1], Exp)
            nc.sync.dma_start(kT_sb[:, 0:128], kT[:, 0:128])
            nc.scalar.dma_start(qT_sb[:, 0:NQB], qT[:, 0:NQB])
            nc.sync.dma_start(kT_sb[:, 128:512], kT[:, 128:512])
            nc.gpsimd.dma_start(vI_sb[:, 0:4], vI[:, 0:4])
            for ck4 in range(4, NCHUNK, 4):
                sl = slice(ck4 * 128, (ck4 + 4) * 128)
                nc.sync.dma_start(kT_sb[:, sl], kT[:, sl])
                nc.gpsimd.dma_start(vI_sb[:, ck4:ck4 + 4], vI[:, ck4:ck4 + 4])
            for b in range(1, NBLK):
                nc.gpsimd.dma_start(qT_sb[:, b * NQB:(b + 1) * NQB],
                                    qT[:, b * NQB:(b + 1) * NQB])
            # Software pipeline, flat across all 8 nq blocks: PV for chunk
            # t is emitted ~3 chunks behind the scores matmuls, SPLIT
            # AROUND the QK pair (PV_h0 before, PV_h1 after), and the
            # queue carries across block boundaries so the PE never
            # drains at a block edge.
            pv_queue = []
            accs = {}

            def emit_pv_h(args, h):
                pt_ap, ck_, bb = args
                first = ck_ == 0
                last = ck_ == NCHUNK - 1
                nc.tensor.matmul(
                    accs[bb][h][:],
                    lhsT=vI_sb[:, ck_, h, :],
                    rhs=pt_ap[:, h * NQB:(h + 1) * NQB],
                    start=first, stop=last)

            def finish_block(bb):
                """Stage block bb's accumulators to SBUF and ship to host.

                The softmax normalize AND the output projection both run
                host-side (fp32 W via BLAS) — the device ships only the
                bf16 numerator accumulators (16x less DMA than fp32
                projection partials) and the fp32 denominator rows.
                Copies split across DVE/ACT to shorten the tail chain."""
                ah0_, ah1_ = accs.pop(bb)
                st = ep_pool.tile([128, NQB], bf16, tag="st")
                nc.vector.tensor_copy(st[0:HD, :], ah0_[0:HD, :])
                nc.scalar.activation(st[HD:128, :], ah1_[0:HD, :], Copy)
                dden = ep_pool.tile([HD + 1, 2 * NQB], f32, tag="dden")
                nc.vector.tensor_copy(dden[HD:HD + 1, 0:NQB],
                                      ah0_[HD:HD + 1, :])
                nc.scalar.activation(dden[HD:HD + 1, NQB:2 * NQB],
                                     ah1_[HD:HD + 1, :], Copy)
                nc.sync.dma_start(dno[bb:bb + 1, :], dden[HD:HD + 1, :])
                nc.scalar.dma_start(stout[bb], st[:])

            for blk in range(NBLK):
                q0 = blk * NQB
                a_h0 = ps_a.tile([HD + 1, NQB], f32, tag="a_h0")
                a_h1 = ps_a.tile([HD + 1, NQB], f32, tag="a_h1")
                accs[blk] = (a_h0, a_h1)
                if blk == 0:
                    for f in range(4):
                        nc.tensor.matmul(
                            (a_h0 if f % 2 == 0 else a_h1)[:],
                            lhsT=zeros_sb[:, 0:HD + 1], rhs=zeros_sb[:],
                            start=False, stop=False)

                for ck in range(NCHUNK):
                    if len(pv_queue) > 2:
                        emit_pv_h(pv_queue[0], 0)
                    # one [128, 1024] scores tile per chunk: h0 in bank 0,
                    # h1 in bank 1, the two matmuls run as a concurrent
                    # row-tiled pair (K=64 at array rows 0 / 64).
                    s_t = ps_s.tile([128, 2 * NQB], f32, tag="s_t")
                    nc.tensor.matmul(
                        s_t[:, 0:NQB],
                        lhsT=kT_sb[0:HD, ck * 128:(ck + 1) * 128],
                        rhs=qT_sb[0:HD, q0:q0 + NQB],
                        start=True, stop=True, tile_position=(0, 0))
                    nc.tensor.matmul(
                        s_t[:, NQB:2 * NQB],
                        lhsT=kT_sb[HD:128, ck * 128:(ck + 1) * 128],
                        rhs=qT_sb[HD:128, q0:q0 + NQB],
                        start=True, stop=True, tile_position=(64, 0))
                    if ck in DVE_CKS:
                        u = pt_pool.tile([128, 2 * NQB], i16, tag="ptu")
                        nc.vector.tensor_scalar(
                            u[:], s_t[:], SCH_A, SCH_B, mult, add)
                        pv_queue.append((u[:].bitcast(bf16), ck, blk))
                    else:
                        pt = pt_pool.tile([128, 2 * NQB], bf16, tag="pt")
                        nc.scalar.activation(pt[:], s_t[:], Exp, scale=0.125)
                        pv_queue.append((pt[:], ck, blk))
                    if len(pv_queue) > 3:
                        e = pv_queue.pop(0)
                        emit_pv_h(e, 1)
                        if e[1] == NCHUNK - 1:
                            finish_block(e[2])
            while pv_queue:
                e = pv_queue.pop(0)
                emit_pv_h(e, 0)
                emit_pv_h(e, 1)
                if e[1] == NCHUNK - 1:
                    finish_block(e[2])

    nc.compile()
    return nc


def _get_compiled():
    global _compiled
    if _compiled is None:
        _compiled = _build_nc()
    return _compiled


def _prep_core_inputs(x, proj_w):
    """Host-side shard + layout per core: core c -> batch c//4, head pair c%4."""
    import ml_dtypes
    bf16 = ml_dtypes.bfloat16

    ins = []
    for c in range(NCORES):
        b, hp = c // 4, c % 4
        sl = slice(128 * hp, 128 * hp + 128)
        qT = np.ascontiguousarray(x[0, b, :, sl].T).astype(bf16)
        kT = np.ascontiguousarray(x[1, b, :, sl].T).astype(bf16)
        v = x[2, b, :, sl]                       # [N, 128]
        vI = np.ones((128, NCHUNK, 2, HD + 1), np.float32)
        vr = v.reshape(NCHUNK, 128, 2, HD)        # [chunk, p, head, m]
        vI[:, :, :, :HD] = vr.transpose(1, 0, 2, 3)
        ins.append({"qT": qT, "kT": kT, "vI": vI.astype(bf16)})
    return ins


def kernel(x, proj_w, proj_b):
    from concourse.bass_utils import run_bass_kernel_spmd

    x = np.asarray(x, dtype=np.float32)
    proj_w = np.asarray(proj_w, dtype=np.float32)
    proj_b = np.asarray(proj_b, dtype=np.float32)

    nc = _get_compiled()
    in_maps = _prep_core_inputs(x, proj_w)
    res = run_bass_kernel_spmd(nc, in_maps, core_ids=list(range(NCORES)))

    out = np.zeros((B, N, D), np.float32)
    for c in range(NCORES):
        r = res.results[c]
        b, hp = c // 4, c % 4
        sl = slice(128 * hp, 128 * hp + 128)
        rec = 1.0 / r["dno"].reshape(NBLK, 2, NQB).transpose(1, 0, 2).reshape(2, N)
        stf = np.asarray(r["stout"]).astype(np.float32)
        stf = stf.transpose(1, 0, 2).reshape(128, N)
        stf[0:HD] *= rec[0][None, :]
        stf[HD:128] *= rec[1][None, :]
        out[b] += stf.T @ proj_w[:, sl].T
    out += proj_b
    return out


# revision 20
# speedup vs baseline: 1.0103x; 1.0103x over previous
"""Multi-head attention + output projection on 8 Trainium2 NeuronCores.

Problem (hardcoded): x [3, 2, 4096, 512] fp32 (q/k/v stacked), proj_w [512, 512],
proj_b [512].  reference = softmax(q k^T / sqrt(64)) v, heads=8, then
out @ proj_w.T + proj_b.

Sharding: B*H = 16 (batch, head) pairs over 8 cores -> each core gets one
batch and one adjacent head PAIR (2 heads = 128 feature dims).  The output
projection is tensor-parallel along the contraction dim: each core computes
its partial y = attn_out_pair @ W[:, pair_dims].T; the host sums the 4
partials per batch and adds the bias.

Device kernel (per core).  All matmul operands are bf16, PSUM accumulation
fp32:
  scores^T[nk, nq] = kT_chunk.T @ qT      (two heads row-tiled in the PE
                                           array: K=64 each at rows 0-63 /
                                           64-127, concurrent)
  P^T: split across two engines per chunk —
    ScalarE chunks:  pt = exp(0.125 * scores^T) -> bf16   (native ACT exp)
    VectorE chunks:  u  = int16(round(A*scores^T + B)); the int16 bit
      pattern IS the bf16 encoding of exp(0.125*s) (Schraudolph trick,
      ~±3% sawtooth, zero-mean calibrated); PV reads u.bitcast(bf16).
  acc[d, nq]  += [V | 1].T @ P^T          (K=128; row 64 = softmax denom)
  y_h[nq, o]   = st_h.T @ W_h  as a row-tiled concurrent pair (h0 rows
                 0-63, h1 rows 64-127), shipped to the host UN-normalized
                 together with the denominator rows; the host applies
                 y = y_h0/den_h0 + y_h1/den_h1 (fp32).  This deletes the
                 den-transpose DMA round trip and the ScalarE/VectorE
                 normalize chain from the device critical path.
"""

import numpy as np

C, B, N, D, H = 3, 2, 4096, 512, 8
HD = 64          # head dim
NCORES = 8
NQB = 512        # nq block width (PSUM bank)
NBLK = N // NQB  # 8 nq blocks
NCHUNK = N // 128  # 32 nk chunks of 128

# Schraudolph constants: bf16 bits of exp(0.125*s) ~ round(A*s + B).
SCH_A = float(16.0 / np.log(2.0))
SCH_B = float(128.0 * 127.0 - 7.36)
# chunks whose exp runs on the DVE (rest on ScalarE); spread evenly.
DVE_CKS = frozenset({2, 4, 6, 9, 11, 14, 16, 19, 21, 24, 26, 28, 30})

_compiled = None


def _build_nc():
    import concourse.bacc as bacc
    import concourse.tile as tile
    from concourse import mybir

    f32 = mybir.dt.float32
    bf16 = mybir.dt.bfloat16
    i16 = mybir.dt.int16
    Exp = mybir.ActivationFunctionType.Exp
    Copy = mybir.ActivationFunctionType.Copy
    mult = mybir.AluOpType.mult
    add = mybir.AluOpType.add

    nc = bacc.Bacc("TRN2", target_bir_lowering=False, debug=False, num_devices=1)

    qT = nc.dram_tensor("qT", [128, N], bf16, kind="ExternalInput").ap()
    kT = nc.dram_tensor("kT", [128, N], bf16, kind="ExternalInput").ap()
    vI = nc.dram_tensor("vI", [128, NCHUNK, 2, HD + 1], bf16, kind="ExternalInput").ap()
    stout = nc.dram_tensor("stout", [NBLK, 128, NQB], bf16,
                           kind="ExternalOutput").ap()
    dno = nc.dram_tensor("dno", [NBLK, 2 * NQB], f32, kind="ExternalOutput").ap()

    with tile.TileContext(nc) as tc:
        with (
            tc.tile_pool(name="const", bufs=1) as const_pool,
            tc.tile_pool(name="pt", bufs=5) as pt_pool,
            tc.tile_pool(name="ep", bufs=3) as ep_pool,
            tc.tile_pool(name="ps_s", bufs=2, space="PSUM") as ps_s,
            tc.tile_pool(name="ps_a", bufs=2, space="PSUM") as ps_a,
        ):
            # resident inputs
            qT_sb = const_pool.tile([128, N], bf16)
            kT_sb = const_pool.tile([128, N], bf16)
            vI_sb = const_pool.tile([128, NCHUNK, 2, HD + 1], bf16)
            # zero weights for the HAM-warming filler matmuls first so the
            # PE can start warming at t~0, then trigger the exp table load
            # while the input DMAs stream in
            zeros_sb = const_pool.tile([128, NQB], mybir.dt.bfloat16)
            nc.vector.memset(zeros_sb[:], 0.0)
            warm = ep_pool.tile([128, 2], f32, tag="warm")
            nc.vector.memset(warm[:], 0.0)
            nc.scalar.activation(warm[:, 1:2], warm[:, 0:1], Exp)
            nc.sync.dma_start(kT_sb[:, 0:128], kT[:, 0:128])
            nc.scalar.dma_start(qT_sb[:, 0:NQB], qT[:, 0:NQB])
            nc.sync.dma_start(kT_sb[:, 128:512], kT[:, 128:512])
            nc.gpsimd.dma_start(vI_sb[:, 0:4], vI[:, 0:4])
            for ck4 in range(4, NCHUNK, 4):
                sl = slice(ck4 * 128, (ck4 + 4) * 128)
                nc.sync.dma_start(kT_sb[:, sl], kT[:, sl])
                nc.gpsimd.dma_start(vI_sb[:, ck4:ck4 + 4], vI[:, ck4:ck4 + 4])
            for b in range(1, NBLK):
                nc.gpsimd.dma_start(qT_sb[:, b * NQB:(b + 1) * NQB],
                                    qT[:, b * NQB:(b + 1) * NQB])
            # Software pipeline, flat across all 8 nq blocks: PV for chunk
            # t is emitted ~3 chunks behind the scores matmuls, SPLIT
            # AROUND the QK pair (PV_h0 before, PV_h1 after), and the
            # queue carries across block boundaries so the PE never
            # drains at a block edge.
            pv_queue = []
            accs = {}

            def emit_pv_h(args, h):
                pt_ap, ck_, bb = args
                first = ck_ == 0
                last = ck_ == NCHUNK - 1
                nc.tensor.matmul(
                    accs[bb][h][:],
                    lhsT=vI_sb[:, ck_, h, :],
                    rhs=pt_ap[:, h * NQB:(h + 1) * NQB],
                    start=first, stop=last)

            def finish_block(bb):
                """Stage block bb's accumulators to SBUF and ship to host.

                The softmax normalize AND the output projection both run
                host-side (fp32 W via BLAS) — the device ships only the
                bf16 numerator accumulators (16x less DMA than fp32
                projection partials) and the fp32 denominator rows.
                Copies split across DVE/ACT to shorten the tail chain."""
                ah0_, ah1_ = accs.pop(bb)
                st = ep_pool.tile([128, NQB], bf16, tag="st")
                nc.vector.tensor_copy(st[0:HD, :], ah0_[0:HD, :])
                nc.scalar.activation(st[HD:128, :], ah1_[0:HD, :], Copy)
                dden = ep_pool.tile([HD + 1, 2 * NQB], f32, tag="dden")
                nc.vector.tensor_copy(dden[HD:HD + 1, 0:NQB],
                                      ah0_[HD:HD + 1, :])
                nc.scalar.activation(dden[HD:HD + 1, NQB:2 * NQB],
                                     ah1_[HD:HD + 1, :], Copy)
                nc.sync.dma_start(dno[bb:bb + 1, :], dden[HD:HD + 1, :])
                nc.scalar.dma_start(stout[bb], st[:])

            for blk in range(NBLK):
                q0 = blk * NQB
                a_h0 = ps_a.tile([HD + 1, NQB], f32, tag="a_h0")
                a_h1 = ps_a.tile([HD + 1, NQB], f32, tag="a_h1")
                accs[blk] = (a_h0, a_h1)
                if blk == 0:
                    for f in range(10):
                        nc.tensor.matmul(
                            (a_h0 if f % 2 == 0 else a_h1)[:],
                            lhsT=zeros_sb[:, 0:HD + 1], rhs=zeros_sb[:],
                            start=False, stop=False)

                for ck in range(NCHUNK):
                    if len(pv_queue) > 2:
                        emit_pv_h(pv_queue[0], 0)
                    # one [128, 1024] scores tile per chunk: h0 in bank 0,
                    # h1 in bank 1, the two matmuls run as a concurrent
                    # row-tiled pair (K=64 at array rows 0 / 64).
                    s_t = ps_s.tile([128, 2 * NQB], f32, tag="s_t")
                    nc.tensor.matmul(
                        s_t[:, 0:NQB],
                        lhsT=kT_sb[0:HD, ck * 128:(ck + 1) * 128],
                        rhs=qT_sb[0:HD, q0:q0 + NQB],
                        start=True, stop=True, tile_position=(0, 0))
                    nc.tensor.matmul(
                        s_t[:, NQB:2 * NQB],
                        lhsT=kT_sb[HD:128, ck * 128:(ck + 1) * 128],
                        rhs=qT_sb[HD:128, q0:q0 + NQB],
                        start=True, stop=True, tile_position=(64, 0))
                    if ck in DVE_CKS:
                        u = pt_pool.tile([128, 2 * NQB], i16, tag="ptu")
                        nc.vector.tensor_scalar(
                            u[:], s_t[:], SCH_A, SCH_B, mult, add)
                        pv_queue.append((u[:].bitcast(bf16), ck, blk))
                    else:
                        pt = pt_pool.tile([128, 2 * NQB], bf16, tag="pt")
                        nc.scalar.activation(pt[:], s_t[:], Exp, scale=0.125)
                        pv_queue.append((pt[:], ck, blk))
                    if len(pv_queue) > 3:
                        e = pv_queue.pop(0)
                        emit_pv_h(e, 1)
                        if e[1] == NCHUNK - 1:
                            finish_block(e[2])
            while pv_queue:
                e = pv_queue.pop(0)
                emit_pv_h(e, 0)
                emit_pv_h(e, 1)
                if e[1] == NCHUNK - 1:
                    finish_block(e[2])

    nc.compile()
    return nc


def _get_compiled():
    global _compiled
    if _compiled is None:
        _compiled = _build_nc()
    return _compiled


def _prep_core_inputs(x, proj_w):
    """Host-side shard + layout per core: core c -> batch c//4, head pair c%4."""
    import ml_dtypes
    bf16 = ml_dtypes.bfloat16

    ins = []
    for c in range(NCORES):
        b, hp = c // 4, c % 4
        sl = slice(128 * hp, 128 * hp + 128)
        qT = np.ascontiguousarray(x[0, b, :, sl].T).astype(bf16)
        kT = np.ascontiguousarray(x[1, b, :, sl].T).astype(bf16)
        v = x[2, b, :, sl]                       # [N, 128]
        vI = np.ones((128, NCHUNK, 2, HD + 1), np.float32)
        vr = v.reshape(NCHUNK, 128, 2, HD)        # [chunk, p, head, m]
        vI[:, :, :, :HD] = vr.transpose(1, 0, 2, 3)
        ins.append({"qT": qT, "kT": kT, "vI": vI.astype(bf16)})
    return ins


def kernel(x, proj_w, proj_b):
    from concourse.bass_utils import run_bass_kernel_spmd

    x = np.asarray(x, dtype=np.float32)
    proj_w = np.asarray(proj_w, dtype=np.float32)
    proj_b = np.asarray(proj_b, dtype=np.float32)

    nc = _get_compiled()
    in_maps = _prep_core_inputs(x, proj_w)
    res = run_bass_kernel_spmd(nc, in_maps, core_ids=list(range(NCORES)))

    out = np.zeros((B, N, D), np.float32)
    for c in range(NCORES):
        r = res.results[c]
        b, hp = c // 4, c % 4
        sl = slice(128 * hp, 128 * hp + 128)
        rec = 1.0 / r["dno"].reshape(NBLK, 2, NQB).transpose(1, 0, 2).reshape(2, N)
        stf = np.asarray(r["stout"]).astype(np.float32)
        stf = stf.transpose(1, 0, 2).reshape(128, N)
        stf[0:HD] *= rec[0][None, :]
        stf[HD:128] *= rec[1][None, :]
        out[b] += stf.T @ proj_w[:, sl].T
    out += proj_b
    return out
